# revision 25
# baseline (speedup 1.0000x reference)
"""Multi-head self-attention on 8 Trainium2 NeuronCores.

Problem: hidden [B=2, S=4096, D=768], H=12 heads x DH=64, fp32.
  q/k/v = x @ Wq/k/v (+bias), per-head softmax(q k^T / 8) @ v, out = ctx @ Wo + bo.

Sharding: data-parallel over batch (2) x tensor-parallel over head groups
(4 groups of 3 heads).  Core cid = b*4 + g gets x[b] and the weight slices
for heads [3g, 3g+3) and returns partial_g = ctx_g @ Wo[192g:192(g+1), :].
The host sums the 4 partials per batch and adds (bv @ Wo + bo) (exact since
bv/bo enter linearly; bq/bk are zero, asserted).

Per-core program (v2 — restructured from the phase-serial baseline):
  phase 1: x -> bf16, transposed via plain matmul-by-identity (bf16, fast,
    keeps PE warm); Q^T/K^T per head as [128, S] with duplicated 64-row
    halves (score matmuls alternate PE row groups); head-2 q/k projections
    column-packed into one matmul chain; V natural + ones column.
  phase 2 (qt-outer): for each q-tile of 512: 3 heads x (scores -> exp ->
    PV accumulate), softmax exp SPLIT between ScalarE (exact LUT exp) and
    VectorE (Schraudolph: i16 = A*s + B bit-viewed as bf16 ~ exp(s/8),
    +-3% prob error that largely cancels in the softmax sum); then
    normalize (reciprocal of the ones-row denominator, K=1 broadcast
    matmul, multiply) and the output projection for this q-tile inline —
    out-proj DMA streams while later q-tiles still compute.
"""

import math
import os
from contextlib import ExitStack

import numpy as np

import concourse.bacc as bacc
import concourse.bass as bass
import concourse.tile as tile
from concourse import mybir
from concourse.masks import make_identity

F32 = mybir.dt.float32
BF16 = mybir.dt.bfloat16
I16 = mybir.dt.int16

# Full problem constants
B, S, D = 2, 4096, 768
H, DH = 12, 64
N_CORES = 8
GROUPS = 4          # tensor-parallel head groups
HL = H // GROUPS    # heads per core = 3
M = HL * DH         # local projection width = 192
SCALE = 1.0 / float(np.sqrt(DH).astype(np.float32))

P = 128             # partitions
QT = 512            # q tile (free dim of score matmuls)
EXPG = 2            # k-chunks per exp group (2*512 fp32 = 2 PSUM banks)

# Schraudolph fast-exp constants: i16 = trunc(A*s + B) bit-viewed as bf16
# approximates exp(SCALE*s).  bf16 bits = 128*(log2(v) + 127); c=0.043
# centers the piecewise-linear error (+-3%), +0.5 compensates truncation.
EXP_A = SCALE * math.log2(math.e) * 128.0
EXP_B = 128.0 * (127.0 - 0.043) + 0.5

# which exp groups (of 16 per (h, qt)) go to the DVE fast-exp path
DVE_GROUPS = frozenset((1, 3, 5, 7, 9, 11, 14))

# A/B experiment toggle: alternate PE row groups for score matmuls (default on)
SCORE_PAIR = os.environ.get("SCORE_PAIR", "1") == "1"
# split PV matmuls into K=64 halves on alternating row groups (dual psum acc)
PV_SPLIT = os.environ.get("PV_SPLIT", "0") == "1"


def emit_attention(ctx: ExitStack, tc: tile.TileContext, out_ap, x_ap,
                   wq_ap, wk_ap, wv_ap, wo_ap, s=S, d=D):
    """out_ap [s, d] f32; x_ap [s, d] f32; wq/wk/wv_ap [d, M]; wo_ap [M, d]."""
    nc = tc.nc
    DC = d // P          # d chunks (6)
    NQ = s // QT         # q tiles (8)
    KC = s // P          # k chunks (32)

    const_pool = ctx.enter_context(tc.tile_pool(name="const", bufs=1))
    identity = const_pool.tile([P, P], BF16)
    make_identity(nc, identity)

    # ---- persistent activation tiles ----
    qk_pool = ctx.enter_context(tc.tile_pool(name="qk", bufs=1))
    qts = [qk_pool.tile([P, s], BF16, tag=f"qt{h}", name=f"qt{h}") for h in range(HL)]
    kts = [qk_pool.tile([P, s], BF16, tag=f"kt{h}", name=f"kt{h}") for h in range(HL)]
    # V natural per head (+ trailing ones col): [128, KC, HL, 65] bf16
    v_all = qk_pool.tile([P, KC, HL, 65], BF16)
    nc.vector.memset(v_all[:, :, :, 64:65], 1.0)

    wo_pool = ctx.enter_context(tc.tile_pool(name="wo", bufs=1))
    wo_a = wo_pool.tile([P, d], BF16)
    wo_b = wo_pool.tile([64, d], BF16)

    # ========== phase 1: transpose x + Q^T/K^T/V ==========
    with ExitStack() as p1:
        w1_pool = p1.enter_context(tc.tile_pool(name="w1", bufs=1))
        wq_f = w1_pool.tile([P, DC, M], F32)
        wk_f = w1_pool.tile([P, DC, M], F32)
        wv_f = w1_pool.tile([P, DC, M], F32)
        wo_af = w1_pool.tile([P, d], F32)
        wo_bf = w1_pool.tile([64, d], F32)
        nc.sync.dma_start(wq_f, wq_ap.rearrange("(c p) m -> p c m", p=P))
        nc.sync.dma_start(wk_f, wk_ap.rearrange("(c p) m -> p c m", p=P))
        nc.sync.dma_start(wv_f, wv_ap.rearrange("(c p) m -> p c m", p=P))
        nc.sync.dma_start(wo_af, wo_ap[0:P, :])
        nc.sync.dma_start(wo_bf, wo_ap[P:M, :])
        nc.vector.tensor_copy(wo_a, wo_af)
        nc.vector.tensor_copy(wo_b, wo_bf)

        wb_pool = p1.enter_context(tc.tile_pool(name="wb", bufs=1))
        wqA = wb_pool.tile([P, DC, P], BF16)    # q heads 0,1
        wkA = wb_pool.tile([P, DC, P], BF16)    # k heads 0,1
        wqkB = wb_pool.tile([P, DC, P], BF16)   # [q head 2 | k head 2]
        wv_t = wb_pool.tile([P, DC, M], BF16)
        nc.vector.tensor_copy(wqA, wq_f[:, :, 0:P])
        nc.vector.tensor_copy(wkA, wk_f[:, :, 0:P])
        nc.vector.tensor_copy(wqkB[:, :, 0:64], wq_f[:, :, P:M])
        nc.vector.tensor_copy(wqkB[:, :, 64:P], wk_f[:, :, P:M])
        nc.vector.tensor_copy(wv_t, wv_f)

        hn_pool = p1.enter_context(tc.tile_pool(name="hn", bufs=8))
        hb_pool = p1.enter_context(tc.tile_pool(name="hb", bufs=6))
        ht_pool = p1.enter_context(tc.tile_pool(name="ht", bufs=3))
        tp_psum = p1.enter_context(
            tc.tile_pool(name="tp", bufs=2, space="PSUM"))
        pj_psum = p1.enter_context(
            tc.tile_pool(name="pj", bufs=4, space="PSUM"))

        for st in range(NQ):            # s tiles of 512
            ss = slice(st * QT, (st + 1) * QT)
            hbs = []
            for j in range(4):
                hn = hn_pool.tile([P, d], F32, tag="hn", name=f"hn{st}_{j}")
                nc.sync.dma_start(hn, x_ap[(st * 4 + j) * P:(st * 4 + j + 1) * P, :])
                hb = hb_pool.tile([P, d], BF16, tag="hb", name=f"hb{st}_{j}")
                nc.vector.tensor_copy(hb, hn)
                hbs.append(hb)
            ht = ht_pool.tile([P, DC, QT], BF16, tag="ht", name="ht")
            for c in range(DC):
                tp = tp_psum.tile([P, QT], F32, tag="tp", name="tp")
                for j in range(4):
                    nc.tensor.matmul(tp[:, j * P:(j + 1) * P],
                                     lhsT=hbs[j][:, c * P:(c + 1) * P],
                                     rhs=identity, start=True, stop=True)
                nc.scalar.copy(ht[:, c, :], tp)
            # Q^T / K^T for this s tile
            psA = pj_psum.tile([P, QT], F32, tag="pj", name="psA")
            for c in range(DC):
                nc.tensor.matmul(psA, lhsT=wqA[:, c, :], rhs=ht[:, c, :],
                                 start=(c == 0), stop=(c == DC - 1))
            nc.scalar.copy(qts[0][0:64, ss], psA[0:64, :])
            nc.scalar.copy(qts[1][64:P, ss], psA[64:P, :])
            psK = pj_psum.tile([P, QT], F32, tag="pj", name="psK")
            for c in range(DC):
                nc.tensor.matmul(psK, lhsT=wkA[:, c, :], rhs=ht[:, c, :],
                                 start=(c == 0), stop=(c == DC - 1))
            nc.scalar.copy(kts[0][0:64, ss], psK[0:64, :])
            nc.scalar.copy(kts[1][64:P, ss], psK[64:P, :])
            psB = pj_psum.tile([P, QT], F32, tag="pj", name="psB")
            for c in range(DC):
                nc.tensor.matmul(psB, lhsT=wqkB[:, c, :], rhs=ht[:, c, :],
                                 start=(c == 0), stop=(c == DC - 1))
            nc.vector.tensor_copy(qts[2][0:64, ss], psB[0:64, :])
            nc.vector.tensor_copy(kts[2][64:P, ss], psB[64:P, :])
            # duplicate q/k halves for this s tile (partition move => DMA)
            nc.sync.dma_start(qts[0][64:P, ss], qts[0][0:64, ss])
            nc.sync.dma_start(qts[1][0:64, ss], qts[1][64:P, ss])
            nc.sync.dma_start(qts[2][64:P, ss], qts[2][0:64, ss])
            nc.sync.dma_start(kts[0][64:P, ss], kts[0][0:64, ss])
            nc.sync.dma_start(kts[1][0:64, ss], kts[1][64:P, ss])
            nc.sync.dma_start(kts[2][0:64, ss], kts[2][64:P, ss])
            # V natural for the 4 s-subchunks
            for j in range(4):
                psV = pj_psum.tile([P, M], F32, tag="pj", name="psV")
                for c in range(DC):
                    nc.tensor.matmul(psV,
                                     lhsT=ht[:, c, j * P:(j + 1) * P],
                                     rhs=wv_t[:, c, :],
                                     start=(c == 0), stop=(c == DC - 1))
                ic = st * 4 + j
                nc.vector.tensor_copy(v_all[:, ic, :, 0:64],
                                      psV.rearrange("p (h e) -> p h e", h=HL))

    # ================= phase 2: attention + out-proj, qt-outer =================
    groups = []
    kc0 = 0
    while kc0 < KC:
        groups.append(list(range(kc0, min(kc0 + EXPG, KC))))
        kc0 += EXPG

    ntiles = [(0, 384), (384, 384)]
    NG = len(groups)
    LAG = 4             # pv trails sc/exp by LAG pipeline units
    with ExitStack() as p2:
        sc_psum = p2.enter_context(
            tc.tile_pool(name="sc", bufs=2, space="PSUM"))
        pv_psum = p2.enter_context(
            tc.tile_pool(name="pv", bufs=1 if PV_SPLIT else 2, space="PSUM"))
        op_psum = p2.enter_context(
            tc.tile_pool(name="op", bufs=2, space="PSUM"))
        pr_pool = p2.enter_context(tc.tile_pool(name="pr", bufs=6))
        cx_pool = p2.enter_context(tc.tile_pool(name="cx", bufs=2))
        rc_pool = p2.enter_context(tc.tile_pool(name="rc", bufs=2))
        bb_pool = p2.enter_context(tc.tile_pool(name="bb", bufs=2))
        cn_pool = p2.enter_context(tc.tile_pool(name="cn", bufs=2))
        ob_pool = p2.enter_context(tc.tile_pool(name="ob", bufs=3))

        ctxns = {}      # qt_i -> (ctxnA, ctxn2, ctxn1s)
        pvs = {}        # (qt_i, h) -> pv psum tile

        def emit_out_proj(qt_i):
            ctxnA, ctxn2, _ = ctxns.pop(qt_i)
            for si in range(QT // P):
                rows = slice(qt_i * QT + si * P, qt_i * QT + (si + 1) * P)
                csl = slice(si * P, (si + 1) * P)
                for (n0, nw) in ntiles:
                    op = op_psum.tile([P, 384], F32, tag="op", name="op")
                    nc.tensor.matmul(op[:, 0:nw], lhsT=ctxnA[:, csl],
                                     rhs=wo_a[:, n0:n0 + nw],
                                     start=True, stop=False)
                    nc.tensor.matmul(op[:, 0:nw], lhsT=ctxn2[:, csl],
                                     rhs=wo_b[:, n0:n0 + nw],
                                     start=False, stop=True)
                    ot = ob_pool.tile([P, 384], F32, tag="ot", name="ot")
                    nc.vector.tensor_copy(ot[:, 0:nw], op[:, 0:nw])
                    nc.sync.dma_start(out_ap[rows, n0:n0 + nw], ot[:, 0:nw])

        def emit_pv(qt_i, h, gi, g, pr):
            if (qt_i, h) not in pvs:
                if PV_SPLIT:
                    pvs[(qt_i, h)] = pv_psum.tile([65, 2, QT], F32, tag="pv",
                                                  name=f"pv{qt_i}_{h}")
                else:
                    pvs[(qt_i, h)] = pv_psum.tile([P, QT], F32, tag="pv",
                                                  name=f"pv{qt_i}_{h}")
            pv = pvs[(qt_i, h)]
            for i, kc in enumerate(g):
                if PV_SPLIT:
                    # two K=64 halves on alternating PE row groups -> they
                    # stream concurrently and LDWEIGHTS pulls ahead
                    nc.tensor.matmul(pv[:, 0, :],
                                     lhsT=v_all[0:64, kc, h, :],
                                     rhs=pr[0:64, i, :],
                                     start=(kc == 0), stop=(kc == KC - 1),
                                     tile_position=(0, 0))
                    nc.tensor.matmul(pv[:, 1, :],
                                     lhsT=v_all[64:P, kc, h, :],
                                     rhs=pr[64:P, i, :],
                                     start=(kc == 0), stop=(kc == KC - 1),
                                     tile_position=(64, 0))
                else:
                    nc.tensor.matmul(pv[0:65, :], lhsT=v_all[:, kc, h, :],
                                     rhs=pr[:, i, :],
                                     start=(kc == 0), stop=(kc == KC - 1))
            if gi != NG - 1:
                return
            # last k-group of this head: evacuate + normalize
            pv = pvs.pop((qt_i, h))
            cx = cx_pool.tile([65, QT], BF16, tag="cx", name="cx")
            if PV_SPLIT:
                nc.scalar.copy(cx, pv[:, 0, :])
                nc.vector.tensor_add(cx, cx, pv[:, 1, :])
            else:
                nc.scalar.copy(cx, pv[0:65, :])
            rc = rc_pool.tile([1, QT], BF16, tag="rc", name="rc")
            with nc.allow_low_precision(reason="softmax denom is O(S)"):
                nc.vector.reciprocal(rc, cx[64:65, :])
            bb = bb_pool.tile([64, QT], BF16, tag="bb", name="bb")
            nc.gpsimd.partition_broadcast(bb, rc)
            ctxnA, ctxn2, ctxn1s = ctxns[qt_i]
            dst = (ctxnA[0:64, :], ctxn1s, ctxn2)[h]
            nc.vector.tensor_mul(dst, cx[0:64, :], bb)
            if h == 1:
                # head 1 into rows 64:128 (partition move => DMA)
                nc.sync.dma_start(ctxnA[64:P, :], ctxn1s)
            elif h == 2:
                op_ready.append(qt_i)

        units = [(qt_i, h, gi, g)
                 for qt_i in range(NQ) for h in range(HL)
                 for gi, g in enumerate(groups)]
        pend = []
        op_ready = []   # qt indices whose out-proj deps are emitted
        op_delay = 0    # units to wait before emitting a ready out-proj
        for qt_i, h, gi, g in units:
            if op_ready:
                if op_delay >= 10:
                    emit_out_proj(op_ready.pop(0))
                    op_delay = 0
                else:
                    op_delay += 1
            if h == 0 and gi == 0:
                qs = slice(qt_i * QT, (qt_i + 1) * QT)
                ctxnA = cn_pool.tile([P, QT], BF16, tag="cnA",
                                     name=f"ctxnA{qt_i}")
                ctxn2 = cn_pool.tile([64, QT], BF16, tag="cn2",
                                     name=f"ctxn2_{qt_i}")
                ctxn1s = cn_pool.tile([64, QT], BF16, tag="cn1s",
                                      name=f"ctxn1s{qt_i}")
                ctxns[qt_i] = (ctxnA, ctxn2, ctxn1s)
            qs = slice(qt_i * QT, (qt_i + 1) * QT)
            q_t, k_t = qts[h], kts[h]
            gl = len(g)
            sc = sc_psum.tile([P, EXPG, QT], F32, tag="sc", name="sc")
            for i, kc in enumerate(g):
                half = (kc % 2) * 64 if SCORE_PAIR else 0
                nc.tensor.matmul(
                    sc[:, i, :],
                    lhsT=k_t[half:half + 64, kc * P:(kc + 1) * P],
                    rhs=q_t[half:half + 64, qs],
                    start=True, stop=True,
                    tile_position=(half, 0))
            pr = pr_pool.tile([P, EXPG, QT], BF16, tag="pr", name="pr")
            if gi in DVE_GROUPS:
                nc.vector.tensor_scalar(
                    pr[:, 0:gl, :].bitcast(I16), sc[:, 0:gl, :],
                    EXP_A, EXP_B,
                    mybir.AluOpType.mult, mybir.AluOpType.add)
            else:
                nc.scalar.activation(
                    pr[:, 0:gl, :], sc[:, 0:gl, :],
                    mybir.ActivationFunctionType.Exp, scale=SCALE)
            pend.append((qt_i, h, gi, g, pr))
            if len(pend) > LAG:
                emit_pv(*pend.pop(0))
        for item in pend:
            emit_pv(*item)
        for qt_i in op_ready:
            emit_out_proj(qt_i)


def build_program(s=S, d=D, reps=1):
    nc = bacc.Bacc("TRN2", target_bir_lowering=False, debug=False,
                   enable_asserts=False, num_devices=N_CORES)
    x_t = nc.dram_tensor("x", [s, d], F32, kind="ExternalInput")
    wq_t = nc.dram_tensor("wq", [d, M], F32, kind="ExternalInput")
    wk_t = nc.dram_tensor("wk", [d, M], F32, kind="ExternalInput")
    wv_t = nc.dram_tensor("wv", [d, M], F32, kind="ExternalInput")
    wo_t = nc.dram_tensor("wo", [M, d], F32, kind="ExternalInput")
    out_t = nc.dram_tensor("out", [s, d], F32, kind="ExternalOutput")
    with tile.TileContext(nc) as tc:
        for _ in range(reps):
            with ExitStack() as ctx:
                emit_attention(ctx, tc, out_t.ap(), x_t.ap(), wq_t.ap(),
                               wk_t.ap(), wv_t.ap(), wo_t.ap(), s=s, d=d)
    nc.compile()
    return nc


_NC_CACHE = {}


def kernel(hidden_states, Wq, bq, Wk, bk, Wv, bv, Wo, bo):
    from concourse.bass_utils import run_bass_kernel_spmd

    hidden_states = np.asarray(hidden_states, dtype=np.float32)
    Wq, Wk, Wv, Wo = (np.asarray(w, dtype=np.float32) for w in (Wq, Wk, Wv, Wo))
    bq, bk, bv, bo = (np.asarray(b_, dtype=np.float32) for b_ in (bq, bk, bv, bo))
    assert float(np.abs(bq).max(initial=0.0)) == 0.0, "nonzero bq unsupported"
    assert float(np.abs(bk).max(initial=0.0)) == 0.0, "nonzero bk unsupported"

    if "nc" not in _NC_CACHE:
        _NC_CACHE["nc"] = build_program()
    nc = _NC_CACHE["nc"]

    in_maps = []
    for cid in range(N_CORES):
        b_i, g = divmod(cid, GROUPS)
        ms = slice(g * M, (g + 1) * M)
        in_maps.append({
            "x": np.ascontiguousarray(hidden_states[b_i]),
            "wq": np.ascontiguousarray(Wq[:, ms]),
            "wk": np.ascontiguousarray(Wk[:, ms]),
            "wv": np.ascontiguousarray(Wv[:, ms]),
            "wo": np.ascontiguousarray(Wo[ms, :]),
        })
    res = run_bass_kernel_spmd(nc, in_maps, core_ids=list(range(N_CORES)))
    # bv and bo enter linearly: ctx = ctx0 + bv  =>  out += bv @ Wo + bo
    host_bias = (bv @ Wo + bo).astype(np.float32)
    out = np.empty((B, S, D), dtype=np.float32)
    for b_i in range(B):
        acc = res.results[b_i * GROUPS + 0]["out"].astype(np.float32)
        for g in range(1, GROUPS):
            acc = acc + res.results[b_i * GROUPS + g]["out"]
        out[b_i] = acc + host_bias
    return out


# revision 27
# speedup vs baseline: 7.3193x; 7.3193x over previous
"""Multi-head self-attention on 8 Trainium2 NeuronCores.

Problem: hidden [B=2, S=4096, D=768], H=12 heads x DH=64, fp32.
  q/k/v = x @ Wq/k/v (+bias), per-head softmax(q k^T / 8) @ v, out = ctx @ Wo + bo.

Sharding: data-parallel over batch (2) x tensor-parallel over head groups
(4 groups of 3 heads).  Core cid = b*4 + g gets x[b] and the weight slices
for heads [3g, 3g+3) and returns partial_g = ctx_g @ Wo[192g:192(g+1), :].
The host sums the 4 partials per batch and adds (bv @ Wo + bo) (exact since
bv/bo enter linearly; bq/bk are zero, asserted).

Per-core program (v2 — restructured from the phase-serial baseline):
  phase 1: x -> bf16, transposed via plain matmul-by-identity (bf16, fast,
    keeps PE warm); Q^T/K^T per head as [128, S] with duplicated 64-row
    halves (score matmuls alternate PE row groups); head-2 q/k projections
    column-packed into one matmul chain; V natural + ones column.
  phase 2 (qt-outer): for each q-tile of 512: 3 heads x (scores -> exp ->
    PV accumulate), softmax exp SPLIT between ScalarE (exact LUT exp) and
    VectorE (Schraudolph: i16 = A*s + B bit-viewed as bf16 ~ exp(s/8),
    +-3% prob error that largely cancels in the softmax sum); then
    normalize (reciprocal of the ones-row denominator, K=1 broadcast
    matmul, multiply) and the output projection for this q-tile inline —
    out-proj DMA streams while later q-tiles still compute.
"""

import math
import os
from contextlib import ExitStack

import numpy as np

import concourse.bacc as bacc
import concourse.bass as bass
import concourse.tile as tile
from concourse import mybir
from concourse.masks import make_identity

F32 = mybir.dt.float32
BF16 = mybir.dt.bfloat16
I16 = mybir.dt.int16

# Full problem constants
B, S, D = 2, 4096, 768
H, DH = 12, 64
N_CORES = 8
GROUPS = 4          # tensor-parallel head groups
HL = H // GROUPS    # heads per core = 3
M = HL * DH         # local projection width = 192
SCALE = 1.0 / float(np.sqrt(DH).astype(np.float32))

P = 128             # partitions
QT = 512            # q tile (free dim of score matmuls)
EXPG = 2            # k-chunks per exp group (2*512 fp32 = 2 PSUM banks)

# Schraudolph fast-exp constants: i16 = trunc(A*s + B) bit-viewed as bf16
# approximates exp(SCALE*s).  bf16 bits = 128*(log2(v) + 127); c=0.043
# centers the piecewise-linear error (+-3%), +0.5 compensates truncation.
EXP_A = SCALE * math.log2(math.e) * 128.0
EXP_B = 128.0 * (127.0 - 0.043) + 0.5

# which exp groups (of 16 per (h, qt)) go to the DVE fast-exp path
DVE_GROUPS = frozenset((1, 3, 5, 7, 9, 11, 14))

# A/B experiment toggle: alternate PE row groups for score matmuls (default on)
SCORE_PAIR = os.environ.get("SCORE_PAIR", "1") == "1"
# split PV matmuls into K=64 halves on alternating row groups (dual psum acc)
PV_SPLIT = os.environ.get("PV_SPLIT", "0") == "1"


def emit_attention(ctx: ExitStack, tc: tile.TileContext, out_ap, x_ap,
                   wq_ap, wk_ap, wv_ap, wo_ap, s=S, d=D):
    """out_ap [s, d] f32; x_ap [s, d] f32; wq/wk/wv_ap [d, M]; wo_ap [M, d]."""
    nc = tc.nc
    DC = d // P          # d chunks (6)
    NQ = s // QT         # q tiles (8)
    KC = s // P          # k chunks (32)

    const_pool = ctx.enter_context(tc.tile_pool(name="const", bufs=1))
    identity = const_pool.tile([P, P], BF16)
    make_identity(nc, identity)

    # ---- persistent activation tiles ----
    qk_pool = ctx.enter_context(tc.tile_pool(name="qk", bufs=1))
    qts = [qk_pool.tile([P, s], BF16, tag=f"qt{h}", name=f"qt{h}") for h in range(HL)]
    kts = [qk_pool.tile([P, s], BF16, tag=f"kt{h}", name=f"kt{h}") for h in range(HL)]
    # V natural per head (+ trailing ones col): [128, KC, HL, 65] bf16
    v_all = qk_pool.tile([P, KC, HL, 65], BF16)
    nc.vector.memset(v_all[:, :, :, 64:65], 1.0)

    wo_pool = ctx.enter_context(tc.tile_pool(name="wo", bufs=1))
    wo_a = wo_pool.tile([P, d], BF16)
    wo_b = wo_pool.tile([64, d], BF16)

    # ========== phase 1: transpose x + Q^T/K^T/V ==========
    with ExitStack() as p1:
        w1_pool = p1.enter_context(tc.tile_pool(name="w1", bufs=1))
        wq_f = w1_pool.tile([P, DC, M], F32)
        wk_f = w1_pool.tile([P, DC, M], F32)
        wv_f = w1_pool.tile([P, DC, M], F32)
        wo_af = w1_pool.tile([P, d], F32)
        wo_bf = w1_pool.tile([64, d], F32)
        nc.sync.dma_start(wq_f, wq_ap.rearrange("(c p) m -> p c m", p=P))
        nc.sync.dma_start(wk_f, wk_ap.rearrange("(c p) m -> p c m", p=P))
        nc.sync.dma_start(wv_f, wv_ap.rearrange("(c p) m -> p c m", p=P))
        nc.sync.dma_start(wo_af, wo_ap[0:P, :])
        nc.sync.dma_start(wo_bf, wo_ap[P:M, :])
        nc.vector.tensor_copy(wo_a, wo_af)
        nc.vector.tensor_copy(wo_b, wo_bf)

        wb_pool = p1.enter_context(tc.tile_pool(name="wb", bufs=1))
        wqA = wb_pool.tile([P, DC, P], BF16)    # q heads 0,1
        wkA = wb_pool.tile([P, DC, P], BF16)    # k heads 0,1
        wqkB = wb_pool.tile([P, DC, P], BF16)   # [q head 2 | k head 2]
        wv_t = wb_pool.tile([P, DC, M], BF16)
        nc.vector.tensor_copy(wqA, wq_f[:, :, 0:P])
        nc.vector.tensor_copy(wkA, wk_f[:, :, 0:P])
        nc.vector.tensor_copy(wqkB[:, :, 0:64], wq_f[:, :, P:M])
        nc.vector.tensor_copy(wqkB[:, :, 64:P], wk_f[:, :, P:M])
        nc.vector.tensor_copy(wv_t, wv_f)

        hn_pool = p1.enter_context(tc.tile_pool(name="hn", bufs=8))
        hb_pool = p1.enter_context(tc.tile_pool(name="hb", bufs=6))
        ht_pool = p1.enter_context(tc.tile_pool(name="ht", bufs=3))
        tp_psum = p1.enter_context(
            tc.tile_pool(name="tp", bufs=2, space="PSUM"))
        pj_psum = p1.enter_context(
            tc.tile_pool(name="pj", bufs=4, space="PSUM"))

        for st in range(NQ):            # s tiles of 512
            ss = slice(st * QT, (st + 1) * QT)
            hbs = []
            for j in range(4):
                hn = hn_pool.tile([P, d], F32, tag="hn", name=f"hn{st}_{j}")
                nc.sync.dma_start(hn, x_ap[(st * 4 + j) * P:(st * 4 + j + 1) * P, :])
                hb = hb_pool.tile([P, d], BF16, tag="hb", name=f"hb{st}_{j}")
                nc.vector.tensor_copy(hb, hn)
                hbs.append(hb)
            ht = ht_pool.tile([P, DC, QT], BF16, tag="ht", name="ht")
            for c in range(DC):
                tp = tp_psum.tile([P, QT], F32, tag="tp", name="tp")
                for j in range(4):
                    nc.tensor.matmul(tp[:, j * P:(j + 1) * P],
                                     lhsT=hbs[j][:, c * P:(c + 1) * P],
                                     rhs=identity, start=True, stop=True)
                nc.scalar.copy(ht[:, c, :], tp)
            # Q^T / K^T for this s tile
            psA = pj_psum.tile([P, QT], F32, tag="pj", name="psA")
            for c in range(DC):
                nc.tensor.matmul(psA, lhsT=wqA[:, c, :], rhs=ht[:, c, :],
                                 start=(c == 0), stop=(c == DC - 1))
            nc.scalar.copy(qts[0][0:64, ss], psA[0:64, :])
            nc.scalar.copy(qts[1][64:P, ss], psA[64:P, :])
            psK = pj_psum.tile([P, QT], F32, tag="pj", name="psK")
            for c in range(DC):
                nc.tensor.matmul(psK, lhsT=wkA[:, c, :], rhs=ht[:, c, :],
                                 start=(c == 0), stop=(c == DC - 1))
            nc.scalar.copy(kts[0][0:64, ss], psK[0:64, :])
            nc.scalar.copy(kts[1][64:P, ss], psK[64:P, :])
            psB = pj_psum.tile([P, QT], F32, tag="pj", name="psB")
            for c in range(DC):
                nc.tensor.matmul(psB, lhsT=wqkB[:, c, :], rhs=ht[:, c, :],
                                 start=(c == 0), stop=(c == DC - 1))
            nc.vector.tensor_copy(qts[2][0:64, ss], psB[0:64, :])
            nc.vector.tensor_copy(kts[2][64:P, ss], psB[64:P, :])
            # duplicate q/k halves for this s tile (partition move => DMA)
            nc.sync.dma_start(qts[0][64:P, ss], qts[0][0:64, ss])
            nc.sync.dma_start(qts[1][0:64, ss], qts[1][64:P, ss])
            nc.sync.dma_start(qts[2][64:P, ss], qts[2][0:64, ss])
            nc.sync.dma_start(kts[0][64:P, ss], kts[0][0:64, ss])
            nc.sync.dma_start(kts[1][0:64, ss], kts[1][64:P, ss])
            nc.sync.dma_start(kts[2][0:64, ss], kts[2][64:P, ss])
            # V natural for the 4 s-subchunks
            for j in range(4):
                psV = pj_psum.tile([P, M], F32, tag="pj", name="psV")
                for c in range(DC):
                    nc.tensor.matmul(psV,
                                     lhsT=ht[:, c, j * P:(j + 1) * P],
                                     rhs=wv_t[:, c, :],
                                     start=(c == 0), stop=(c == DC - 1))
                ic = st * 4 + j
                nc.vector.tensor_copy(v_all[:, ic, :, 0:64],
                                      psV.rearrange("p (h e) -> p h e", h=HL))

    # ================= phase 2: attention + out-proj, qt-outer =================
    groups = []
    kc0 = 0
    while kc0 < KC:
        groups.append(list(range(kc0, min(kc0 + EXPG, KC))))
        kc0 += EXPG

    ntiles = [(0, 384), (384, 384)]
    NG = len(groups)
    LAG = 4             # pv trails sc/exp by LAG pipeline units
    with ExitStack() as p2:
        sc_psum = p2.enter_context(
            tc.tile_pool(name="sc", bufs=2, space="PSUM"))
        pv_psum = p2.enter_context(
            tc.tile_pool(name="pv", bufs=1 if PV_SPLIT else 2, space="PSUM"))
        op_psum = p2.enter_context(
            tc.tile_pool(name="op", bufs=2, space="PSUM"))
        pr_pool = p2.enter_context(tc.tile_pool(name="pr", bufs=6))
        cx_pool = p2.enter_context(tc.tile_pool(name="cx", bufs=2))
        rc_pool = p2.enter_context(tc.tile_pool(name="rc", bufs=2))
        bb_pool = p2.enter_context(tc.tile_pool(name="bb", bufs=2))
        cn_pool = p2.enter_context(tc.tile_pool(name="cn", bufs=2))
        ob_pool = p2.enter_context(tc.tile_pool(name="ob", bufs=3))

        ctxns = {}      # qt_i -> (ctxnA, ctxn2, ctxn1s)
        pvs = {}        # (qt_i, h) -> pv psum tile

        def emit_out_proj(qt_i):
            ctxnA, ctxn2, _ = ctxns.pop(qt_i)
            for si in range(QT // P):
                rows = slice(qt_i * QT + si * P, qt_i * QT + (si + 1) * P)
                csl = slice(si * P, (si + 1) * P)
                for (n0, nw) in ntiles:
                    op = op_psum.tile([P, 384], F32, tag="op", name="op")
                    nc.tensor.matmul(op[:, 0:nw], lhsT=ctxnA[:, csl],
                                     rhs=wo_a[:, n0:n0 + nw],
                                     start=True, stop=False)
                    nc.tensor.matmul(op[:, 0:nw], lhsT=ctxn2[:, csl],
                                     rhs=wo_b[:, n0:n0 + nw],
                                     start=False, stop=True)
                    ot = ob_pool.tile([P, 384], F32, tag="ot", name="ot")
                    nc.vector.tensor_copy(ot[:, 0:nw], op[:, 0:nw])
                    nc.sync.dma_start(out_ap[rows, n0:n0 + nw], ot[:, 0:nw])

        def emit_pv(qt_i, h, gi, g, pr):
            if (qt_i, h) not in pvs:
                if PV_SPLIT:
                    pvs[(qt_i, h)] = pv_psum.tile([65, 2, QT], F32, tag="pv",
                                                  name=f"pv{qt_i}_{h}")
                else:
                    pvs[(qt_i, h)] = pv_psum.tile([P, QT], F32, tag="pv",
                                                  name=f"pv{qt_i}_{h}")
            pv = pvs[(qt_i, h)]
            for i, kc in enumerate(g):
                if PV_SPLIT:
                    # two K=64 halves on alternating PE row groups -> they
                    # stream concurrently and LDWEIGHTS pulls ahead
                    nc.tensor.matmul(pv[:, 0, :],
                                     lhsT=v_all[0:64, kc, h, :],
                                     rhs=pr[0:64, i, :],
                                     start=(kc == 0), stop=(kc == KC - 1),
                                     tile_position=(0, 0))
                    nc.tensor.matmul(pv[:, 1, :],
                                     lhsT=v_all[64:P, kc, h, :],
                                     rhs=pr[64:P, i, :],
                                     start=(kc == 0), stop=(kc == KC - 1),
                                     tile_position=(64, 0))
                else:
                    nc.tensor.matmul(pv[0:65, :], lhsT=v_all[:, kc, h, :],
                                     rhs=pr[:, i, :],
                                     start=(kc == 0), stop=(kc == KC - 1))
            if gi != NG - 1:
                return
            # last k-group of this head: evacuate + normalize
            pv = pvs.pop((qt_i, h))
            cx = cx_pool.tile([65, QT], BF16, tag="cx", name="cx")
            if PV_SPLIT:
                nc.scalar.copy(cx, pv[:, 0, :])
                nc.vector.tensor_add(cx, cx, pv[:, 1, :])
            else:
                nc.scalar.copy(cx, pv[0:65, :])
            rc = rc_pool.tile([1, QT], BF16, tag="rc", name="rc")
            with nc.allow_low_precision(reason="softmax denom is O(S)"):
                nc.vector.reciprocal(rc, cx[64:65, :])
            bb = bb_pool.tile([64, QT], BF16, tag="bb", name="bb")
            nc.gpsimd.partition_broadcast(bb, rc)
            ctxnA, ctxn2, ctxn1s = ctxns[qt_i]
            dst = (ctxnA[0:64, :], ctxn1s, ctxn2)[h]
            nc.vector.tensor_mul(dst, cx[0:64, :], bb)
            if h == 1:
                # head 1 into rows 64:128 (partition move => DMA)
                nc.sync.dma_start(ctxnA[64:P, :], ctxn1s)
            elif h == 2:
                op_ready.append(qt_i)

        units = [(qt_i, h, gi, g)
                 for qt_i in range(NQ) for h in range(HL)
                 for gi, g in enumerate(groups)]
        pend = []
        op_ready = []   # qt indices whose out-proj deps are emitted
        op_delay = 0    # units to wait before emitting a ready out-proj
        for qt_i, h, gi, g in units:
            if op_ready:
                if op_delay >= 10:
                    emit_out_proj(op_ready.pop(0))
                    op_delay = 0
                else:
                    op_delay += 1
            if h == 0 and gi == 0:
                qs = slice(qt_i * QT, (qt_i + 1) * QT)
                ctxnA = cn_pool.tile([P, QT], BF16, tag="cnA",
                                     name=f"ctxnA{qt_i}")
                ctxn2 = cn_pool.tile([64, QT], BF16, tag="cn2",
                                     name=f"ctxn2_{qt_i}")
                ctxn1s = cn_pool.tile([64, QT], BF16, tag="cn1s",
                                      name=f"ctxn1s{qt_i}")
                ctxns[qt_i] = (ctxnA, ctxn2, ctxn1s)
            qs = slice(qt_i * QT, (qt_i + 1) * QT)
            q_t, k_t = qts[h], kts[h]
            gl = len(g)
            sc = sc_psum.tile([P, EXPG, QT], F32, tag="sc", name="sc")
            for i, kc in enumerate(g):
                half = (kc % 2) * 64 if SCORE_PAIR else 0
                nc.tensor.matmul(
                    sc[:, i, :],
                    lhsT=k_t[half:half + 64, kc * P:(kc + 1) * P],
                    rhs=q_t[half:half + 64, qs],
                    start=True, stop=True,
                    tile_position=(half, 0))
            pr = pr_pool.tile([P, EXPG, QT], BF16, tag="pr", name="pr")
            if gi in DVE_GROUPS:
                nc.vector.tensor_scalar(
                    pr[:, 0:gl, :].bitcast(I16), sc[:, 0:gl, :],
                    EXP_A, EXP_B,
                    mybir.AluOpType.mult, mybir.AluOpType.add)
            else:
                nc.scalar.activation(
                    pr[:, 0:gl, :], sc[:, 0:gl, :],
                    mybir.ActivationFunctionType.Exp, scale=SCALE)
            pend.append((qt_i, h, gi, g, pr))
            if len(pend) > LAG:
                emit_pv(*pend.pop(0))
        for item in pend:
            emit_pv(*item)
        for qt_i in op_ready:
            emit_out_proj(qt_i)


def build_program(s=S, d=D, reps=1):
    nc = bacc.Bacc("TRN2", target_bir_lowering=False, debug=False,
                   enable_asserts=False, num_devices=N_CORES)
    x_t = nc.dram_tensor("x", [s, d], F32, kind="ExternalInput")
    wq_t = nc.dram_tensor("wq", [d, M], F32, kind="ExternalInput")
    wk_t = nc.dram_tensor("wk", [d, M], F32, kind="ExternalInput")
    wv_t = nc.dram_tensor("wv", [d, M], F32, kind="ExternalInput")
    wo_t = nc.dram_tensor("wo", [M, d], F32, kind="ExternalInput")
    out_t = nc.dram_tensor("out", [s, d], F32, kind="ExternalOutput")
    with tile.TileContext(nc) as tc:
        for _ in range(reps):
            with ExitStack() as ctx:
                emit_attention(ctx, tc, out_t.ap(), x_t.ap(), wq_t.ap(),
                               wk_t.ap(), wv_t.ap(), wo_t.ap(), s=s, d=d)
    nc.compile()
    return nc


_NC_CACHE = {}


def kernel(hidden_states, Wq, bq, Wk, bk, Wv, bv, Wo, bo):
    from concourse.bass_utils import run_bass_kernel_spmd

    hidden_states = np.asarray(hidden_states, dtype=np.float32)
    Wq, Wk, Wv, Wo = (np.asarray(w, dtype=np.float32) for w in (Wq, Wk, Wv, Wo))
    bq, bk, bv, bo = (np.asarray(b_, dtype=np.float32) for b_ in (bq, bk, bv, bo))
    assert float(np.abs(bq).max(initial=0.0)) == 0.0, "nonzero bq unsupported"
    assert float(np.abs(bk).max(initial=0.0)) == 0.0, "nonzero bk unsupported"

    if "nc" not in _NC_CACHE:
        _NC_CACHE["nc"] = build_program()
    nc = _NC_CACHE["nc"]

    in_maps = []
    for cid in range(N_CORES):
        b_i, g = divmod(cid, GROUPS)
        ms = slice(g * M, (g + 1) * M)
        in_maps.append({
            "x": np.ascontiguousarray(hidden_states[b_i]),
            "wq": np.ascontiguousarray(Wq[:, ms]),
            "wk": np.ascontiguousarray(Wk[:, ms]),
            "wv": np.ascontiguousarray(Wv[:, ms]),
            "wo": np.ascontiguousarray(Wo[ms, :]),
        })
    res = run_bass_kernel_spmd(nc, in_maps, core_ids=list(range(N_CORES)))
    # bv and bo enter linearly: ctx = ctx0 + bv  =>  out += bv @ Wo + bo
    host_bias = (bv @ Wo + bo).astype(np.float32)
    out = np.empty((B, S, D), dtype=np.float32)
    for b_i in range(B):
        acc = res.results[b_i * GROUPS + 0]["out"].astype(np.float32)
        for g in range(1, GROUPS):
            acc = acc + res.results[b_i * GROUPS + g]["out"]
        out[b_i] = acc + host_bias
    return out
